# revision 9
# baseline (speedup 1.0000x reference)
"""Trainium2 Bass kernel for nn_AttentionHead (additive/Bahdanau attention).

reference:
    kt = einsum('bkh,oh->bko', x_key, w1)          # (B, NK, H)
    qt = einsum('bqh,oh->bqo', x_query, w2)        # (B, NQ, H)
    prod[b,q,k] = sum_h v[h] * tanh(kt[b,k,h] + qt[b,q,h])
    out = log_softmax(prod, axis=-1)               # (B, NQ, NK)

Shapes: B=4, NQ=256, NK=512, H=256.  8 NeuronCores, data-parallel over
(B x NQ/2): core c handles b = c//2 and a 128-row slice of NQ.

Per-core dataflow:
  - host marshals ONE packed fp32 input (128, 4352): transposed xk, xq,
    w1, w2 plus the "ediag" stationaries (for each (h_tile, j in 0..31) a
    (128,32) matrix, zero except column j = v[h_tile*128 : +128]).
    Single DMA -> single queue semaphore (walrus only supports one sync
    wait per instruction, so every instruction must need at most one new
    semaphore).
  - PE: ktT[o_t] (128, 512) = w1T.T @ xkT       (o on partitions, k free)
        qtT[o_t] (128, 128) = w2T.T @ xqT       (o on partitions, q free)
  - DVE: S[h_t][:, q*512:+512] = ktT[h_t] + qtT[h_t][:, q]   (2x mode)
  - ACT: T = tanh(S) in large (128, 4096) instructions -> bf16 (this is
    the kernel bottleneck: 134M tanh elements live on ScalarE only)
  - PE:  prod[q, :] += ediag[h_t, q%32].T @ T[h_t][:, q]  via col-tiled
    matmuls (tile_position=(0, 32j)) accumulating into one PSUM bank
    (q on partitions, k free)
  - log_softmax along free axis: out = prod - ln(sum_k exp(prod));
    |prod| <= sum|v| ~ 8 so skipping max-subtraction is safe in fp32.

Wait-budget bookkeeping (1 sync wait max per instruction):
  - everything downstream of the DMA sees one DMA queue semaphore
  - ACT tanh(g) would need waits on DVE (input) AND PE (buffer WAR from
    group g-2's matmuls). An ACT nop with an explicit sync dep on the
    last matmul of group g-2 absorbs the PE wait first.
"""

import sys

sys.path.insert(0, "/opt/trn_rl_repo")

import numpy as np

import concourse.bass as bass
import concourse.mybir as mybir
from concourse import tile
from concourse.tile_rust import add_dep_helper
from concourse.bass_utils import run_bass_kernel_spmd

F32 = mybir.dt.float32
BF16 = mybir.dt.bfloat16
AF = mybir.ActivationFunctionType
ALU = mybir.AluOpType

B, NQ, NK, H = 4, 256, 512, 256
NCORES = 8
QPC = (B * NQ) // NCORES  # 128 q rows per core
GROUP = 8                 # q's per pipeline group
NGROUPS = QPC // GROUP    # 16

OFF_XKT = 0
OFF_XQT = 1024
OFF_W1T = 1280
OFF_W2T = 1792
OFF_ED = 2304
PACKED_F = 4352


def build_program(split=True):
    nc = bass.Bass()

    pk_d = nc.dram_tensor("packed", (128, PACKED_F), F32, kind="ExternalInput")
    out_d = nc.dram_tensor("out", (QPC, NK), F32, kind="ExternalOutput")

    with tile.TileContext(nc) as tc:
        with (
            tc.tile_pool(name="const", bufs=1) as cpool,
            tc.tile_pool(name="sadd", bufs=2) as spool,
            tc.tile_pool(name="tanh", bufs=2) as tpool,
            tc.tile_pool(name="ppre", bufs=2, space="PSUM") as ppool,
            tc.tile_pool(name="prod", bufs=1, space="PSUM") as prodpool,
        ):
            packed = cpool.tile([128, PACKED_F], F32, tag="packed")
            nc.sync.dma_start(packed[:], pk_d[:])

            def xkT(i):
                return packed[:, OFF_XKT + i * NK:OFF_XKT + (i + 1) * NK]

            def xqT(i):
                return packed[:, OFF_XQT + i * QPC:OFF_XQT + (i + 1) * QPC]

            def w1T(i, o):
                return packed[:, OFF_W1T + i * 256 + o * 128:OFF_W1T + i * 256 + (o + 1) * 128]

            def w2T(i, o):
                return packed[:, OFF_W2T + i * 256 + o * 128:OFF_W2T + i * 256 + (o + 1) * 128]

            # v-diag stationaries -> bf16 (ACT does the cast; keeps the
            # main-loop matmuls' deps on the single ACT semaphore)
            ed_bf = cpool.tile([128, 2 * 32 * 32], BF16, tag="ed_bf")
            nc.scalar.copy(ed_bf[:], packed[:, OFF_ED:OFF_ED + 2048])

            # ---- ktT / qtT ---------------------------------------------------
            ktT_sb = [cpool.tile([128, NK], F32, tag=f"ktT{o}", name=f"ktT{o}")
                      for o in range(2)]
            qtT_sb = [cpool.tile([128, QPC], F32, tag=f"qtT{o}", name=f"qtT{o}")
                      for o in range(2)]
            for o_t in range(2):
                pk = ppool.tile([128, NK], F32, tag="pk", name="pk")
                for h_t in range(2):
                    nc.tensor.matmul(
                        pk[:], w1T(h_t, o_t), xkT(h_t),
                        start=(h_t == 0), stop=(h_t == 1),
                    )
                nc.vector.tensor_copy(ktT_sb[o_t][:], pk[:])
            for o_t in range(2):
                pq = ppool.tile([128, QPC], F32, tag="pq", name="pq")
                for h_t in range(2):
                    nc.tensor.matmul(
                        pq[:], w2T(h_t, o_t), xqT(h_t),
                        start=(h_t == 0), stop=(h_t == 1),
                    )
                nc.vector.tensor_copy(qtT_sb[o_t][:], pq[:])

            # ---- main loop ---------------------------------------------------
            prod = prodpool.tile([128, NK], F32)
            last_mm_of_group = {}
            for g in range(NGROUPS):
                S = [spool.tile([128, GROUP * NK], F32, tag=f"S{i}", name=f"S{i}")
                     for i in range(2)]
                for ql in range(GROUP):
                    q = g * GROUP + ql
                    for h_t in range(2):
                        nc.vector.tensor_scalar(
                            S[h_t][:, ql * NK:(ql + 1) * NK],
                            ktT_sb[h_t][:],
                            qtT_sb[h_t][:, q:q + 1],
                            None,
                            op0=ALU.add,
                        )
                # Absorb the T-slot WAR (PE readers of group g-2) into an ACT
                # nop so the tanh itself needs only the DVE wait.
                nop = None
                if g - 2 in last_mm_of_group:
                    nop = nc.scalar.nop(hint="absorb_pe_wait")
                    add_dep_helper(nop.ins, last_mm_of_group[g - 2].ins,
                                   True, "absorb PE wait before tanh")
                T = [tpool.tile([128, GROUP * NK], BF16, tag=f"T{i}", name=f"T{i}")
                     for i in range(2)]
                for h_t in range(2):
                    act = nc.scalar.activation(T[h_t][:], S[h_t][:], AF.Tanh)
                    if nop is not None:
                        add_dep_helper(act.ins, nop.ins, False,
                                       "order tanh after absorber nop")
                mm = None
                for ql in range(GROUP):
                    q = g * GROUP + ql
                    j = (q // 32) * 32
                    jj = q % 32
                    for h_t in range(2):
                        mm = nc.tensor.matmul(
                            prod[j:j + 32, :],
                            ed_bf[:, h_t * 1024 + jj * 32: h_t * 1024 + jj * 32 + 32],
                            T[h_t][:, ql * NK:(ql + 1) * NK],
                            start=(jj == 0 and h_t == 0),
                            stop=(jj == 31 and h_t == 1),
                            tile_position=(0, j),
                        )
                last_mm_of_group[g] = mm

            # ---- log_softmax over k (free axis) ------------------------------
            expt = cpool.tile([128, NK], F32, tag="expt")
            sumexp = cpool.tile([128, 1], F32, tag="sumexp")
            nc.scalar.activation(expt[:], prod[:], AF.Exp, accum_out=sumexp[:])
            lse = cpool.tile([128, 1], F32, tag="lse")
            nc.scalar.activation(lse[:], sumexp[:], AF.Ln)
            neg_lse = cpool.tile([128, 1], F32, tag="neg_lse")
            nc.vector.tensor_scalar_mul(neg_lse[:], lse[:], -1.0)
            out_sb = cpool.tile([128, NK], F32, tag="out_sb")
            nc.scalar.activation(
                out_sb[:], prod[:], AF.Identity, bias=neg_lse[:, 0:1]
            )
            nc.sync.dma_start(out_d[:], out_sb[:])

    if split:
        split_multi_waits(nc)
    return nc


def split_multi_waits(nc):
    """walrus codegen accepts at most one sync wait per instruction; move
    extra waits onto same-engine NoOps inserted immediately before."""
    n = 0
    for fn in nc.m.functions:
        for blk in fn.blocks:
            new_insts = []
            for inst in blk.instructions:
                si = inst.sync_info
                if si is not None and len(si.on_wait) > 1:
                    waits = list(si.on_wait)
                    for w in waits[:-1]:
                        nop = mybir.InstNoOp(name=f"WSPLIT-{n}", ins=[], outs=[])
                        n += 1
                        nop.engine = inst.engine
                        nop.sync_info = mybir.SyncInfo(on_wait=[w], on_update=[])
                        new_insts.append(nop)
                    inst.sync_info = mybir.SyncInfo(
                        on_wait=[waits[-1]], on_update=list(si.on_update)
                    )
                new_insts.append(inst)
            if n:
                blk.instructions = new_insts
    return n


def audit_waits(nc, max_waits=1):
    """Every instruction must carry <= max_waits sync waits (walrus limit)."""
    bad = []
    for fn in nc.m.functions:
        for blk in fn.blocks:
            for inst in blk.instructions:
                si = inst.sync_info
                if si is not None and len(si.on_wait) > max_waits:
                    bad.append((inst.name, type(inst).__name__,
                                [w.ant_name for w in si.on_wait]))
    return bad


def make_in_maps(x_query, x_key, w1, w2, v):
    x_query = np.asarray(x_query, dtype=np.float32)
    x_key = np.asarray(x_key, dtype=np.float32)
    w1 = np.asarray(w1, dtype=np.float32)
    w2 = np.asarray(w2, dtype=np.float32)
    v = np.asarray(v, dtype=np.float32).reshape(H)

    w1T = np.ascontiguousarray(w1.T)  # (h_in, o)
    w2T = np.ascontiguousarray(w2.T)

    # ediag[p, h_t*1024 + j*32 + c] = v[h_t*128 + p] if c == j else 0
    ed = np.zeros((128, 2, 32, 32), dtype=np.float32)
    for h_t in range(2):
        for j in range(32):
            ed[:, h_t, j, j] = v[h_t * 128:(h_t + 1) * 128]
    ed = ed.reshape(128, 2 * 32 * 32)

    in_maps = []
    for c in range(NCORES):
        b = c // 2
        q0 = (c % 2) * QPC
        xqT = np.ascontiguousarray(x_query[b, q0:q0 + QPC, :].T)  # (H, 128)
        xkT = np.ascontiguousarray(x_key[b].T)                    # (H, 512)
        packed = np.concatenate(
            [
                xkT[:128], xkT[128:],
                xqT[:128], xqT[128:],
                w1T[:128], w1T[128:],
                w2T[:128], w2T[128:],
                ed,
            ],
            axis=1,
        )
        assert packed.shape == (128, PACKED_F)
        in_maps.append({"packed": np.ascontiguousarray(packed)})
    return in_maps


_prog_cache = {}


def kernel(x_query, x_key, w1, w2, v):
    if "nc" not in _prog_cache:
        _prog_cache["nc"] = build_program()
    nc = _prog_cache["nc"]
    in_maps = make_in_maps(x_query, x_key, w1, w2, v)
    res = run_bass_kernel_spmd(nc, in_maps, list(range(NCORES)))
    out = np.empty((B, NQ, NK), dtype=np.float32)
    for c in range(NCORES):
        b = c // 2
        q0 = (c % 2) * QPC
        out[b, q0:q0 + QPC, :] = res.results[c]["out"]
    return out


if __name__ == "__main__":
    nc = build_program()
    bad = audit_waits(nc)
    if bad:
        print(f"{len(bad)} instructions exceed the 1-wait budget:")
        for name, ty, waits in bad[:20]:
            print(" ", name, ty, waits)
    else:
        print("wait audit OK: all instructions <= 1 sync wait")
